# revision 21
# baseline (speedup 1.0000x reference)
"""3-layer GAT (GATRefiner) on 8 Trainium2 NeuronCores via Bass/Tile.

Strategy (destination-sharded):
  - Nodes are partitioned across 8 cores by destination ownership (6250 each).
  - Per layer: a sharded dense phase computes table rows [hp | alpha_src | alpha_dst]
    for owned nodes, an AllGather replicates the table, then each core processes its
    own destination blocks (128 dsts): edge rows are fetched with dma_gather
    (int16 indices, split-table trick), attention weights computed on-chip, and
    aggregation done with one-hot matmuls accumulating in PSUM.
  - Self-loops are handled as an identity-matmul chunk per block (no gather).
  - Softmax has no max-subtraction (scores are provably small for this model) and
    normalization is applied after aggregation (denominator accumulated alongside).
"""

import math
import numpy as np
import ml_dtypes

N = 50000
E = 800000
IN_DIM = 128
C = 32
H = 4
SLOPE = 0.2
NUM_CLASSES = 2
NCORES = 8
P = 128

bf16 = ml_dtypes.bfloat16
DEBUG = False
DBG_BLK = 0


def _ceil(a, b):
    return -(-a // b) * b


def _wrap16(seg):
    """dma_gather idx layout: [16, n//16] wrapped, replicated x8 -> [128, n//16]."""
    n = len(seg)
    w = seg.reshape(n // 16, 16).T
    return np.tile(w, (8, 1))


def preprocess(edge_index):
    """Build per-core, per-block gather indices and local-dst arrays."""
    src = np.asarray(edge_index[0], dtype=np.int64)
    dst = np.asarray(edge_index[1], dtype=np.int64)
    own = N // NCORES                      # 6250
    nblk = (own + P - 1) // P              # 49
    halfN = N // 2                         # 25000

    order = np.argsort(dst, kind="stable")
    src_s, dst_s = src[order], dst[order]

    # per (core, block) edge lists
    blocks = [[None] * nblk for _ in range(NCORES)]
    maxn0 = maxn1 = 0
    for c in range(NCORES):
        for b in range(nblk):
            g0 = c * own + b * P
            g1 = min(c * own + (b + 1) * P, (c + 1) * own)
            lo = np.searchsorted(dst_s, g0)
            hi = np.searchsorted(dst_s, g1)
            s = src_s[lo:hi]
            d = dst_s[lo:hi] - g0
            lo_mask = s < halfN
            s0, d0 = s[lo_mask], d[lo_mask]
            s1, d1 = s[~lo_mask] - halfN, d[~lo_mask]
            blocks[c][b] = (s0, d0, s1, d1)
            maxn0 = max(maxn0, len(s0))
            maxn1 = max(maxn1, len(s1))

    n0p = max(_ceil(maxn0, P), P)
    n1p = max(_ceil(maxn1, P), P)
    slots = n0p + n1p
    ch = slots // P

    IDX = np.zeros((NCORES, nblk, P, slots // 16), dtype=np.int16)
    DSTL = np.full((NCORES, nblk, P, ch), 999.0, dtype=np.float32)
    for c in range(NCORES):
        for b in range(nblk):
            s0, d0, s1, d1 = blocks[c][b]
            seg0 = np.zeros(n0p, dtype=np.int16)
            seg0[: len(s0)] = s0
            seg1 = np.zeros(n1p, dtype=np.int16)
            seg1[: len(s1)] = s1
            IDX[c, b, :, : n0p // 16] = _wrap16(seg0)
            IDX[c, b, :, n0p // 16 :] = _wrap16(seg1)
            dl = np.full(slots, 999.0, dtype=np.float32)
            dl[: len(d0)] = d0
            dl[n0p : n0p + len(d1)] = d1
            DSTL[c, b] = dl.reshape(ch, P).T
    return IDX, DSTL, n0p, n1p, ch, nblk, own


def build_program(n0p, n1p, ch, nblk, own):
    import concourse.bass as bass
    import concourse.tile as tile
    import concourse.mybir as mybir
    from concourse import bacc
    from concourse.library_config import mlp
    from contextlib import ExitStack

    f32 = mybir.dt.float32
    b16 = mybir.dt.bfloat16
    i16 = mybir.dt.int16
    halfN = N // 2
    slots = n0p + n1p
    HC = H * C                       # 128
    D1 = HC + 2 * H                  # 136 table cols for layers 1-2
    ROW = 256                        # padded bf16 row (512B)
    D3 = C + 2                       # 34 table cols layer 3
    ROW3 = 128                       # padded bf16 row (256B)

    nc = bacc.Bacc("TRN2", target_bir_lowering=False, num_swdge_queues=4)

    # ---- inputs ----
    t_xT1 = nc.dram_tensor("xT1", [P, own], f32, kind="ExternalInput")
    t_W1p = nc.dram_tensor("W1p", [P, D1], f32, kind="ExternalInput")
    t_W2p = nc.dram_tensor("W2p", [P, D1], b16, kind="ExternalInput")
    t_W3p = nc.dram_tensor("W3p", [P, D3], b16, kind="ExternalInput")
    t_IDX = nc.dram_tensor("IDX", [nblk, P, slots // 16], i16, kind="ExternalInput")
    t_DSTL = nc.dram_tensor("DSTL", [nblk, P, ch], f32, kind="ExternalInput")
    t_IOTA = nc.dram_tensor("IOTA", [P, P], b16, kind="ExternalInput")
    t_IDENT = nc.dram_tensor("IDENT", [P, P], b16, kind="ExternalInput")
    t_B1 = nc.dram_tensor("B1", [P, HC], f32, kind="ExternalInput")
    t_B2 = nc.dram_tensor("B2", [P, HC], f32, kind="ExternalInput")
    t_B3 = nc.dram_tensor("B3", [P, C], f32, kind="ExternalInput")
    t_WC = nc.dram_tensor("WC", [P, NUM_CLASSES * C], f32, kind="ExternalInput")
    t_BC = nc.dram_tensor("BC", [P, NUM_CLASSES], f32, kind="ExternalInput")

    t_OUTH = nc.dram_tensor("OUTH", [own, C], f32, kind="ExternalOutput")
    t_OUTN = nc.dram_tensor("OUTN", [own, NUM_CLASSES], f32, kind="ExternalOutput")
    if DEBUG:
        hcD = H * C
        slotsD = n0p + n1p
        t_DT = nc.dram_tensor("DBG_T", [N, ROW], b16, kind="ExternalOutput")
        t_DHG = nc.dram_tensor("DBG_HG", [P, (slotsD // P) * ROW], b16, kind="ExternalOutput")
        t_DSC = nc.dram_tensor("DBG_SC", [P, (slotsD // P) * H], f32, kind="ExternalOutput")
        t_DW = nc.dram_tensor("DBG_W", [P, (slotsD // P) * H], f32, kind="ExternalOutput")
        t_DADE = nc.dram_tensor("DBG_ADE", [P, (slotsD // P) * H], f32, kind="ExternalOutput")
        t_DOUT = nc.dram_tensor("DBG_OUT", [P, hcD], f32, kind="ExternalOutput")
        t_DRW = nc.dram_tensor("DBG_RW", [P, (slotsD // P) * hcD], b16, kind="ExternalOutput")
        t_DAGG = nc.dram_tensor("DBG_AGG", [P, hcD], f32, kind="ExternalOutput")
        t_DDEN = nc.dram_tensor("DBG_DEN", [P, H], f32, kind="ExternalOutput")

    rg = [list(range(NCORES))]

    with tile.TileContext(nc) as tc, ExitStack() as ctx:
        dram = ctx.enter_context(tc.tile_pool(name="dram", bufs=1, space="DRAM"))
        consts = ctx.enter_context(tc.tile_pool(name="consts", bufs=1))
        dsp = ctx.enter_context(tc.tile_pool(name="dsp", bufs=3))           # dense sbuf
        dps = ctx.enter_context(tc.tile_pool(name="dps", bufs=1, space="PSUM"))
        idxp = ctx.enter_context(tc.tile_pool(name="idxp", bufs=3))
        hgp = ctx.enter_context(tc.tile_pool(name="hgp", bufs=2))
        sp = ctx.enter_context(tc.tile_pool(name="sp", bufs=3))             # small per-block
        s01p = ctx.enter_context(tc.tile_pool(name="s01p", bufs=4))
        tps = ctx.enter_context(tc.tile_pool(name="tps", bufs=2, space="PSUM"))
        aggps = ctx.enter_context(tc.tile_pool(name="aggps", bufs=1, space="PSUM"))
        epip = ctx.enter_context(tc.tile_pool(name="epip", bufs=3))

        nc.gpsimd.load_library(mlp)

        # persistent tiles
        T12 = dram.tile([N, ROW], b16)            # gathered table, layers 1-2
        T3 = dram.tile([N, ROW3], b16)
        Town12 = dram.tile([own, ROW], b16)
        Town3 = dram.tile([own, ROW3], b16)
        xTown = consts.tile([P, own], b16)        # feature-major own activations

        IOTA = consts.tile([P, P], b16)
        IDENT = consts.tile([P, P], b16)
        nc.sync.dma_start(IOTA[:], t_IOTA[:])
        nc.sync.dma_start(IDENT[:], t_IDENT[:])
        W1p = consts.tile([P, D1], f32)
        W2p = consts.tile([P, D1], b16)
        W3p = consts.tile([P, D3], b16)
        nc.sync.dma_start(W1p[:], t_W1p[:])
        nc.sync.dma_start(W2p[:], t_W2p[:])
        nc.sync.dma_start(W3p[:], t_W3p[:])
        B1 = consts.tile([P, HC], f32)
        B2 = consts.tile([P, HC], f32)
        B3 = consts.tile([P, C], f32)
        WC = consts.tile([P, NUM_CLASSES * C], f32)
        nc.sync.dma_start(B1[:], t_B1[:])
        nc.sync.dma_start(B2[:], t_B2[:])
        nc.sync.dma_start(B3[:], t_B3[:])
        nc.sync.dma_start(WC[:], t_WC[:])

        def dense_phase(layer):
            """own-node rows of the table: [hp | alS | alD] via x @ [W|Wa_s|Wa_d]."""
            D = D3 if layer == 3 else D1
            ROWL = ROW3 if layer == 3 else ROW
            Town = Town3 if layer == 3 else Town12
            Wp = {1: W1p, 2: W2p, 3: W3p}[layer]
            for b in range(nblk):
                nb = min(P, own - b * P)
                ps = dps.tile([P, D], f32, space="PSUM")
                if layer == 1:
                    lhsT = dsp.tile([P, P], f32, tag="lhsT1")
                    nc.sync.dma_start(lhsT[:, :nb], t_xT1[:, b * P : b * P + nb])
                else:
                    lhsT = dsp.tile([P, P], b16, tag="lhsT2")
                    nc.vector.tensor_copy(lhsT[:, :nb], xTown[:, b * P : b * P + nb])
                nc.tensor.matmul(ps[:nb], lhsT[:, :nb], Wp[:], start=True, stop=True)
                rowt = dsp.tile([P, ROWL], b16, tag="rowt")
                nc.scalar.copy(rowt[:nb, :D], ps[:nb])
                nc.sync.dma_start(Town[b * P : b * P + nb, :], rowt[:nb, :])

        def allgather(layer):
            Town = Town3 if layer == 3 else Town12
            T = T3 if layer == 3 else T12
            nc.gpsimd.collective_compute(
                "AllGather",
                bass.mybir.AluOpType.bypass,
                replica_groups=rg,
                ins=[Town[:].opt()],
                outs=[T[:].opt()],
            )

        def grind(layer):
            heads = 1 if layer == 3 else H
            cdim = C
            hc = heads * cdim                       # 128 or 32
            D = D3 if layer == 3 else D1
            ROWL = ROW3 if layer == 3 else ROW
            T = T3 if layer == 3 else T12
            elem = ROWL
            for b in range(nblk):
                nb = min(P, own - b * P)
                g0 = b * P
                # block-local data
                idxt = idxp.tile([P, slots // 16], i16)
                nc.sync.dma_start(idxt[:], t_IDX[b])
                dstl = sp.tile([P, ch], f32, tag="dstl")
                nc.sync.dma_start(dstl[:], t_DSTL[b])
                R = sp.tile([P, ROWL], b16, tag="R")
                Town = Town3 if layer == 3 else Town12
                nc.sync.dma_start(R[:nb], Town[g0 : g0 + nb, :])

                hg = hgp.tile([P, ch * elem], b16)
                hgv = hg[:].rearrange("p (k d) -> p k d", d=elem)
                q = (2 * b) % 4
                nc.gpsimd.dma_gather(
                    out_ap=hgv[:, : n0p // P, :],
                    in_ap=T[:halfN, :],
                    idxs_ap=idxt[:, : n0p // 16],
                    num_idxs=n0p, num_idxs_reg=n0p, elem_size=elem,
                    single_packet=False, queue_num=q,
                )
                nc.gpsimd.dma_gather(
                    out_ap=hgv[:, n0p // P :, :],
                    in_ap=T[halfN:, :],
                    idxs_ap=idxt[:, n0p // 16 :],
                    num_idxs=n1p, num_idxs_reg=n1p, elem_size=elem,
                    single_packet=False, queue_num=(q + 1) % 4,
                )

                # attention pre-softmax scores and one-hot matrices
                alDe = tps.tile([P, ch * heads], f32, space="PSUM", tag="alDe", bufs=1)
                psA_t = aggps.tile([P, hc], f32, space="PSUM", tag="psA", bufs=2)
                psD_t = aggps.tile([P, heads], f32, space="PSUM", tag="psD", bufs=2)
                psA = psA_t[:]
                psD = psD_t[:]
                nc.vector.memset(psA, 0.0)
                nc.vector.memset(psD, 0.0)
                s01s = []
                for cidx in range(ch):
                    s01 = s01p.tile([P, P], b16, tag="s01", bufs=ch + 2)
                    nc.vector.tensor_scalar(
                        out=s01[:], in0=IOTA[:], scalar1=dstl[:, cidx : cidx + 1],
                        scalar2=None, op0=bass.mybir.AluOpType.is_equal,
                    )
                    s01s.append(s01)
                    s01T_ps = tps.tile([P, P], b16, space="PSUM", tag="s01T_ps", bufs=1)
                    nc.tensor.transpose(s01T_ps[:], s01[:], IDENT[:])
                    s01T = s01p.tile([P, P], b16, tag="s01T", bufs=3)
                    nc.scalar.copy(s01T[:], s01T_ps[:])
                    nc.tensor.matmul(
                        alDe[:, cidx * heads : (cidx + 1) * heads],
                        s01T[:], R[:, hc + heads : hc + 2 * heads],
                        start=True, stop=True,
                    )

                # scores -> weights
                sc = sp.tile([P, ch * heads], f32, tag="sc")
                nc.vector.tensor_tensor(
                    out=sc[:].rearrange("p (k h) -> p k h", h=heads),
                    in0=hgv[:, :, hc : hc + heads],
                    in1=alDe[:].rearrange("p (k h) -> p k h", h=heads),
                    op=bass.mybir.AluOpType.add,
                )
                sc2 = sp.tile([P, ch * heads], f32, tag="sc2")
                nc.vector.tensor_scalar(out=sc2[:], in0=sc[:], scalar1=SLOPE,
                                        scalar2=None, op0=bass.mybir.AluOpType.mult)
                nc.vector.tensor_tensor(out=sc[:], in0=sc[:], in1=sc2[:],
                                        op=bass.mybir.AluOpType.max)
                w = sp.tile([P, ch * heads], b16, tag="w")
                nc.scalar.activation(w[:], sc[:], bass.mybir.ActivationFunctionType.Exp)

                # weighted messages
                rhsW = hgp.tile([P, ch * hc], b16, tag="rhsW")
                wv = w[:].rearrange("p (k h) -> p k h", h=heads)
                nc.vector.tensor_tensor(
                    out=rhsW[:].rearrange("p (k h d) -> p k h d", h=heads, d=cdim),
                    in0=hgv[:, :, :hc].rearrange("p k (h d) -> p k h d", d=cdim),
                    in1=wv.unsqueeze(3).broadcast_to([P, ch, heads, cdim]),
                    op=bass.mybir.AluOpType.mult,
                )

                # self-loop weights
                wself_s = sp.tile([P, heads], f32, tag="wself_s")
                nc.vector.tensor_tensor(
                    out=wself_s[:], in0=R[:, hc : hc + heads],
                    in1=R[:, hc + heads : hc + 2 * heads],
                    op=bass.mybir.AluOpType.add,
                )
                wself2 = sp.tile([P, heads], f32, tag="wself2")
                nc.vector.tensor_scalar(out=wself2[:], in0=wself_s[:], scalar1=SLOPE,
                                        scalar2=None, op0=bass.mybir.AluOpType.mult)
                nc.vector.tensor_tensor(out=wself_s[:], in0=wself_s[:], in1=wself2[:],
                                        op=bass.mybir.AluOpType.max)
                wself = sp.tile([P, heads], b16, tag="wself")
                nc.scalar.activation(wself[:], wself_s[:], bass.mybir.ActivationFunctionType.Exp)
                rhsWs = sp.tile([P, hc], b16, tag="rhsWs")
                nc.vector.tensor_tensor(
                    out=rhsWs[:].rearrange("p (h d) -> p h d", d=cdim),
                    in0=R[:, :hc].rearrange("p (h d) -> p h d", d=cdim),
                    in1=wself[:].unsqueeze(2).broadcast_to([P, heads, cdim]),
                    op=bass.mybir.AluOpType.mult,
                )

                # aggregation
                rhsWv = rhsW[:].rearrange("p (k d) -> p k d", d=hc)
                for cidx in range(ch):
                    nc.tensor.matmul(psA, s01s[cidx][:], rhsWv[:, cidx, :],
                                     start=False, stop=False, skip_group_check=True)
                    nc.tensor.matmul(psD, s01s[cidx][:], wv[:, cidx, :],
                                     start=False, stop=False, skip_group_check=True)
                nc.tensor.matmul(psA, IDENT[:], rhsWs[:], start=False, stop=True, skip_group_check=True)
                nc.tensor.matmul(psD, IDENT[:], wself[:], start=False, stop=True, skip_group_check=True)

                if DEBUG and layer == 1 and b == DBG_BLK:
                    nc.sync.dma_start(t_DRW[:], rhsW[:])
                    dagg = sp.tile([P, hc], f32, tag="dbgagg")
                    nc.vector.tensor_copy(dagg[:], psA)
                    nc.sync.dma_start(t_DAGG[:], dagg[:])
                    nc.sync.dma_start(t_DHG[:], hg[:])
                    nc.sync.dma_start(t_DSC[:], sc[:])
                    dw = sp.tile([P, ch * heads], f32, tag="dbgw")
                    nc.vector.tensor_copy(dw[:], w[:])
                    nc.sync.dma_start(t_DW[:], dw[:])
                    dade = sp.tile([P, ch * heads], f32, tag="dbgade")
                    nc.vector.tensor_copy(dade[:], alDe[:])
                    nc.sync.dma_start(t_DADE[:], dade[:])
                    dden = sp.tile([P, heads], f32, tag="dbgden")
                    nc.vector.tensor_copy(dden[:], psD)
                    nc.sync.dma_start(t_DDEN[:], dden[:])
                # epilogue: normalize + bias (+ELU / +classifier)
                rec = epip.tile([P, heads], f32, tag="rec")
                nc.vector.reciprocal(rec[:], psD)
                outb = epip.tile([P, hc], f32, tag="outb")
                nc.vector.tensor_tensor(
                    out=outb[:].rearrange("p (h d) -> p h d", d=cdim),
                    in0=psA.rearrange("p (h d) -> p h d", d=cdim),
                    in1=rec[:].unsqueeze(2).broadcast_to([P, heads, cdim]),
                    op=bass.mybir.AluOpType.mult,
                )
                if layer < 3:
                    Bt = B1 if layer == 1 else B2
                    nc.vector.tensor_tensor(out=outb[:], in0=outb[:], in1=Bt[:],
                                            op=bass.mybir.AluOpType.add)
                    # ELU = max(x,0)-1 + exp(min(x,0))
                    mn = epip.tile([P, hc], f32, tag="mn")
                    nc.vector.tensor_scalar(out=mn[:], in0=outb[:], scalar1=0.0,
                                            scalar2=None, op0=bass.mybir.AluOpType.min)
                    nc.scalar.activation(mn[:], mn[:], bass.mybir.ActivationFunctionType.Exp)
                    mx = epip.tile([P, hc], f32, tag="mx")
                    nc.vector.tensor_scalar(out=mx[:], in0=outb[:], scalar1=0.0,
                                            scalar2=-1.0, op0=bass.mybir.AluOpType.max,
                                            op1=bass.mybir.AluOpType.add)
                    x2 = epip.tile([P, hc], b16, tag="x2")
                    nc.vector.tensor_tensor(out=x2[:], in0=mx[:], in1=mn[:],
                                            op=bass.mybir.AluOpType.add)
                    if DEBUG and layer == 1 and b == DBG_BLK:
                        dout = epip.tile([P, hc], f32, tag="dbgout")
                        nc.vector.tensor_copy(dout[:], x2[:])
                        nc.sync.dma_start(t_DOUT[:], dout[:])
                    # transpose into xTown
                    xt_ps = tps.tile([P, P], b16, space="PSUM", tag="xt_ps", bufs=1)
                    nc.tensor.transpose(xt_ps[:], x2[:], IDENT[:])
                    nc.scalar.copy(xTown[:, g0 : g0 + nb], xt_ps[:, :nb])
                else:
                    nc.vector.tensor_tensor(out=outb[:], in0=outb[:], in1=B3[:],
                                            op=bass.mybir.AluOpType.add)
                    nc.sync.dma_start(t_OUTH[g0 : g0 + nb, :], outb[:nb, :])
                    no = epip.tile([P, NUM_CLASSES], f32, tag="no")
                    tmp = epip.tile([P, C], f32, tag="tmp")
                    for j in range(NUM_CLASSES):
                        nc.vector.tensor_tensor(out=tmp[:], in0=outb[:],
                                                in1=WC[:, j * C : (j + 1) * C],
                                                op=bass.mybir.AluOpType.mult)
                        nc.vector.reduce_sum(no[:, j : j + 1], tmp[:],
                                             axis=bass.mybir.AxisListType.X)
                    bc = epip.tile([P, NUM_CLASSES], f32, tag="bc")
                    nc.sync.dma_start(bc[:], t_BC[:])
                    nc.vector.tensor_tensor(
                        out=no[:], in0=no[:], in1=bc[:],
                        op=bass.mybir.AluOpType.add,
                    )
                    nc.sync.dma_start(t_OUTN[g0 : g0 + nb, :], no[:nb, :])

        for layer in (1, 2, 3):
            dense_phase(layer)
            allgather(layer)
            if DEBUG and layer == 1:
                nc.sync.dma_start(t_DT[:], T12[:])
            grind(layer)

    nc.compile()
    return nc


def make_host_tensors(inputs, IDX, DSTL, core):
    """Per-core in_map."""
    x = np.asarray(inputs["x"], np.float32)

    def wpack(W, a_s, a_d):
        W = np.asarray(W, np.float32)
        a_s = np.asarray(a_s, np.float32)
        a_d = np.asarray(a_d, np.float32)
        heads = a_s.shape[0]
        cdim = a_s.shape[1]
        Was = np.zeros((W.shape[0], heads), np.float32)
        Wad = np.zeros((W.shape[0], heads), np.float32)
        for h in range(heads):
            Was[:, h] = W[:, h * cdim : (h + 1) * cdim] @ a_s[h]
            Wad[:, h] = W[:, h * cdim : (h + 1) * cdim] @ a_d[h]
        return np.concatenate([W, Was, Wad], axis=1)

    W1p = wpack(inputs["W1"], inputs["a1_src"], inputs["a1_dst"])
    W2p = wpack(inputs["W2"], inputs["a2_src"], inputs["a2_dst"]).astype(bf16)
    W3p = wpack(inputs["W3"], inputs["a3_src"], inputs["a3_dst"]).astype(bf16)

    iota = np.tile(np.arange(P, dtype=np.float32), (P, 1)).astype(bf16)
    ident = np.eye(P, dtype=np.float32).astype(bf16)
    B1 = np.tile(np.asarray(inputs["b1"], np.float32), (P, 1))
    B2 = np.tile(np.asarray(inputs["b2"], np.float32), (P, 1))
    B3 = np.tile(np.asarray(inputs["b3"], np.float32), (P, 1))
    Wc = np.asarray(inputs["Wc"], np.float32)             # [C, 2]
    WC = np.concatenate([np.tile(Wc[:, j], (P, 1)) for j in range(NUM_CLASSES)], axis=1)
    BC = np.tile(np.asarray(inputs["bc"], np.float32).reshape(1, NUM_CLASSES), (P, 1))

    return {
        "xT1": np.ascontiguousarray(x.T[:, core * (N // NCORES) : (core + 1) * (N // NCORES)]),
        "W1p": W1p, "W2p": W2p, "W3p": W3p,
        "IDX": IDX[core], "DSTL": DSTL[core],
        "IOTA": iota, "IDENT": ident,
        "B1": B1, "B2": B2, "B3": B3, "WC": WC, "BC": BC,
    }


def kernel_run(inputs, trace=False):
    from concourse.bass_utils import run_bass_kernel_spmd
    import time

    t0 = time.time()
    IDX, DSTL, n0p, n1p, ch, nblk, own = preprocess(np.asarray(inputs["edge_index"]))
    t1 = time.time()
    nc = build_program(n0p, n1p, ch, nblk, own)
    t2 = time.time()
    in_maps = [make_host_tensors(inputs, IDX, DSTL, c) for c in range(NCORES)]
    res = run_bass_kernel_spmd(nc, in_maps, core_ids=list(range(NCORES)), trace=trace)
    t3 = time.time()
    print(f"[kernel] preprocess {t1-t0:.1f}s build+compile {t2-t1:.1f}s run {t3-t2:.1f}s")
    node_out = np.concatenate([res.results[c]["OUTN"] for c in range(NCORES)], axis=0)
    link_emb = np.concatenate([res.results[c]["OUTH"] for c in range(NCORES)], axis=0)
    return (node_out, link_emb), res


def kernel(**inputs):
    out, _ = kernel_run(inputs, trace=False)
    return out
